# revision 25
# baseline (speedup 1.0000x reference)
import os
import sys

for _p in ("/opt/trn_rl_repo", "/root/.axon_site/_ro/trn_rl_repo"):
    if os.path.isdir(_p) and _p not in sys.path:
        sys.path.insert(0, _p)

import numpy as np
import concourse.bacc as bacc
import concourse.mybir as mybir
import concourse.tile as tile
from concourse import bass_utils

B, N, T, F = 8, 128, 2048, 32
L, H = 5, 64

FP32 = mybir.dt.float32
FP32R = mybir.dt.float32r
FP16 = mybir.dt.float16

TT = 256          # t-steps per x tile
HALO = 4          # max_lag - 1
CHUNK = 16        # t-steps per output chunk
NTILES = T // TT  # 8
NCHUNKS = TT // CHUNK  # 16 per tile

X_TILE_FREE = (TT + HALO) * F  # 8320
Y_CHUNK_FREE = CHUNK * H       # 1024

_CACHE = {}
LAST_RESULTS = None


def _build_nc():
    nc = bacc.Bacc("TRN2", target_bir_lowering=False, debug=False)
    x_d = nc.dram_tensor("x", (N, T * F), FP16, kind="ExternalInput").ap()
    at_d = nc.dram_tensor("at", (N, L * N), FP16, kind="ExternalInput").ap()
    wblk_d = nc.dram_tensor("wblk", (128, 256), FP16, kind="ExternalInput").ap()
    bvec_d = nc.dram_tensor("bvec", (128, 1), FP32, kind="ExternalInput").ap()
    zeros_d = nc.dram_tensor("zeros", (N, HALO * F), FP16, kind="ExternalInput").ap()
    ident_d = nc.dram_tensor("ident", (128, 128), FP32, kind="ExternalInput").ap()
    y_d = nc.dram_tensor("y", (N, T * H), FP16, kind="ExternalOutput").ap()

    gelu = mybir.ActivationFunctionType.Gelu

    with tile.TileContext(nc) as tc:
        with (
            tc.tile_pool(name="consts", bufs=1) as consts,
            tc.tile_pool(name="xpool", bufs=2) as xpool,
            tc.tile_pool(name="aggpool", bufs=3) as aggpool,
            tc.tile_pool(name="aggtpool", bufs=3) as aggtpool,
            tc.tile_pool(name="ypool", bufs=12) as ypool,
            tc.tile_pool(name="pagg", bufs=2, space="PSUM") as pagg,
            tc.tile_pool(name="pt", bufs=2, space="PSUM") as pt,
            tc.tile_pool(name="py", bufs=2, space="PSUM") as py,
        ):
            at_sb = consts.tile((N, L * N), FP16)
            w_sb = consts.tile((128, 256), FP16)
            bvec_sb = consts.tile((128, 1), FP32)
            ident_sb = consts.tile((128, 128), FP32R)
            nc.sync.dma_start(out=at_sb, in_=at_d)
            nc.sync.dma_start(out=w_sb, in_=wblk_d)
            nc.sync.dma_start(out=bvec_sb, in_=bvec_d)
            nc.sync.dma_start(out=ident_sb, in_=ident_d.bitcast(FP32R))

            x_tiles = {}
            agg_of = {}
            aggt_of = {}
            TOTAL = NTILES * NCHUNKS

            def emit_xload(ti):
                x_tile = xpool.tile((N, X_TILE_FREE), FP16)
                x_tiles[ti] = x_tile
                t0 = ti * TT
                if ti == 0:
                    nc.sync.dma_start(out=x_tile[:, 0 : HALO * F], in_=zeros_d)
                    src0 = x_d[:, 0 : TT * F]
                    sl = TT * F // 8
                    for s in range(8):
                        nc.sync.dma_start(
                            out=x_tile[:, HALO * F + s * sl : HALO * F + (s + 1) * sl],
                            in_=src0[:, s * sl : (s + 1) * sl],
                        )
                else:
                    src = x_d[:, (t0 - HALO) * F : (t0 + TT) * F]
                    sl = X_TILE_FREE // 8
                    for s in range(8):
                        nc.sync.dma_start(
                            out=x_tile[:, s * sl : (s + 1) * sl],
                            in_=src[:, s * sl : (s + 1) * sl],
                        )

            def emit_s1(g):
                ti, c = divmod(g, NCHUNKS)
                x_tile = x_tiles[ti]
                psum_agg = pagg.tile((N, 512), FP32)
                for lag in range(L):
                    off = (HALO + CHUNK * c - lag) * F
                    nc.tensor.matmul(
                        psum_agg,
                        at_sb[:, lag * N : (lag + 1) * N],
                        x_tile[:, off : off + 512],
                        start=(lag == 0),
                        stop=(lag == L - 1),
                    )
                sbuf_agg = aggpool.tile((N, 512), FP32R)
                nc.vector.tensor_copy(sbuf_agg, psum_agg.bitcast(FP32R))
                agg_of[g] = sbuf_agg

            def emit_tr(g):
                sbuf_agg = agg_of.pop(g)
                psum_t = pt.tile((128, 512), FP32R)
                for s in range(4):
                    nc.tensor.transpose(
                        psum_t[:, s * 128 : (s + 1) * 128],
                        sbuf_agg[:, s * 128 : (s + 1) * 128],
                        ident_sb,
                    )
                sbuf_aggt = aggtpool.tile((N, 512), FP16)
                nc.vector.tensor_copy(sbuf_aggt, psum_t.bitcast(FP32))
                aggt_of[g] = sbuf_aggt

            def emit_s2(g):
                sbuf_aggt = aggt_of.pop(g)
                psum_y = py.tile((N, Y_CHUNK_FREE), FP32)
                for r in range(2):
                    nc.tensor.matmul(
                        psum_y[:, r * 512 : (r + 1) * 512],
                        w_sb[:, r * 128 : (r + 1) * 128],
                        sbuf_aggt,
                        start=True,
                        stop=True,
                    )
                sbuf_y = ypool.tile((N, Y_CHUNK_FREE), FP16)
                nc.scalar.activation(sbuf_y, psum_y, func=gelu, bias=bvec_sb)
                nc.sync.dma_start(
                    out=y_d[:, g * Y_CHUNK_FREE : (g + 1) * Y_CHUNK_FREE], in_=sbuf_y
                )

            emit_xload(0)
            for g in range(TOTAL + 2):
                if g < TOTAL:
                    ti, c = divmod(g, NCHUNKS)
                    emit_s1(g)
                    if c == 0 and ti + 1 < NTILES:
                        emit_xload(ti + 1)
                if 1 <= g <= TOTAL:
                    emit_tr(g - 1)
                if g >= 2:
                    emit_s2(g - 2)
    nc.compile()
    return nc


def kernel(x, A_list, W, b):
    global LAST_RESULTS
    x = np.asarray(x, np.float32)
    A_list = np.asarray(A_list, np.float32)
    W = np.asarray(W, np.float32)
    b = np.asarray(b, np.float32)

    if "nc" not in _CACHE:
        _CACHE["nc"] = _build_nc()
    nc = _CACHE["nc"]

    wblk = np.zeros((128, 256), np.float16)
    for q in range(4):
        wblk[q * F : (q + 1) * F, q * H : (q + 1) * H] = W.T.astype(np.float16)
    bvec = np.ascontiguousarray(np.tile(b, 2)[:, None].astype(np.float32))
    zeros = np.zeros((N, HALO * F), np.float16)
    ident = np.eye(128, dtype=np.float32)

    in_maps = []
    for c in range(B):
        in_maps.append(
            {
                "x": x[c].reshape(N, T * F).astype(np.float16),
                "at": np.ascontiguousarray(
                    A_list[c].transpose(2, 0, 1).reshape(N, L * N)
                ).astype(np.float16),
                "wblk": wblk,
                "bvec": bvec,
                "zeros": zeros,
                "ident": ident,
            }
        )

    trace = bool(os.environ.get("KERNEL_TRACE"))
    res = bass_utils.run_bass_kernel_spmd(
        nc, in_maps, core_ids=list(range(B)), trace=trace
    )
    LAST_RESULTS = res
    outs = []
    for c in range(B):
        arr = np.asarray(res.results[c]["y"])
        arr6 = arr.reshape(2, 64, 128, 2, 4, 128)
        yb = (
            np.transpose(arr6, (5, 2, 4, 3, 0, 1))
            .reshape(N, T, H)
            .astype(np.float32)
        )
        outs.append(yb)
    return np.stack(outs)
